# revision 16
# baseline (speedup 1.0000x reference)
"""Trainium2 Bass kernel for nn_AutoFeedBack (GRU warmup + autoregressive decode).

Single-core persistent kernel: all weights live in SBUF; the 1024-wide GRU
hidden state stays on-chip across all 4496 sequential steps.

Math (keras GRUCell, reset_after=True; biases are zero in this problem):
    mh = h @ R            (PSUM, unit-major: 24 M-tiles of 128)
    mx = x @ W (+ b)      (warmup: batched per block; AR: per-step K=5 matmul)
    z, r = sigmoid(mx_zr + mh_zr)
    hh   = tanh(mx_h + r * mh_h)
    h'   = z*h + (1-z)*hh

Hardware rule (verified empirically): PSUM accumulation groups must be
CONSECUTIVE in PE program order — interleaving matmuls of different groups
corrupts fp32 results. All loops are therefore column-group-outer.
"""
import numpy as np

UNITS = 1024
OUT_STEPS = 400
F = 4
SEQ = 4496
TW = 4096                 # warmup steps
U3 = 3 * UNITS
KC = UNITS // 128         # 8 K-chunks
MC = 24                   # M tiles of the R matvec
BLK = 32                  # warmup block (even; PSUM column count)
UARB = 28                 # AR dynamic-loop block steps (even)
NARB = 14                 # AR dynamic blocks -> 392 steps
ARTAIL = 7                # 392 + 7 = 399 AR steps
AR0 = TW + 1              # first AR input column (4097)

_cache = {}


def _build(wdt_name: str, dense_bias: float, rt_np, wb_np, dsb_np):
    import concourse.mybir as mybir
    import concourse.tile as tile
    from concourse import bacc
    from concourse.bass import ds

    fdt = mybir.dt.float32
    wdt = mybir.dt.bfloat16 if wdt_name == "bf16" else fdt
    AF = mybir.ActivationFunctionType
    OP = mybir.AluOpType

    nc = bacc.Bacc("TRN2", target_bir_lowering=False, debug=False, num_devices=1)
    # weights are baked into the NEFF (inline) — only xt crosses the host
    # boundary per call
    r_d = nc.inline_tensor(rt_np, name="r_t").ap()
    wb_d = nc.inline_tensor(wb_np, name="wb_t").ap()
    dw_d = nc.inline_tensor(dsb_np, name="dw_t").ap()
    xt_d = nc.dram_tensor("xt_t", [5, SEQ], wdt, kind="ExternalInput").ap()
    out_d = nc.dram_tensor("preds", [1, OUT_STEPS], fdt, kind="ExternalOutput").ap()

    ZCOLS = max(BLK, UARB)

    with tile.TileContext(nc) as tc:
        r_sb = nc.alloc_sbuf_tensor("r_sb", [128, KC * MC * 128], wdt).ap()
        wb_sb = nc.alloc_sbuf_tensor("wb_sb", [5, U3], wdt).ap()
        xt_sb = nc.alloc_sbuf_tensor("xt_sb", [5, SEQ], wdt).ap()
        dw_sb = nc.alloc_sbuf_tensor("dw_sb", [128, KC], wdt).ap()
        hb = [
            nc.alloc_sbuf_tensor("h_ping", [128, KC], wdt).ap(),
            nc.alloc_sbuf_tensor("h_pong", [128, KC], wdt).ap(),
        ]
        mx_sb = nc.alloc_sbuf_tensor("mx_sb", [128, MC, BLK], fdt).ap()
        zr_pre = nc.alloc_sbuf_tensor("zr_pre", [128, 16], fdt).ap()
        zr_s = nc.alloc_sbuf_tensor("zr_s", [128, 16], fdt).ap()
        t1 = nc.alloc_sbuf_tensor("t1", [128, 8], fdt).ap()
        t2 = nc.alloc_sbuf_tensor("t2", [128, 8], fdt).ap()
        hh = nc.alloc_sbuf_tensor("hh", [128, 8], fdt).ap()
        dd = nc.alloc_sbuf_tensor("dd", [128, 8], fdt).ap()
        ee = nc.alloc_sbuf_tensor("ee", [128, 8], fdt).ap()
        pr = nc.alloc_sbuf_tensor("pr", [1, OUT_STEPS], fdt).ap()

        def r_tile(k, c):
            off = (k * MC + c) * 128
            return r_sb[:, off : off + 128]

        def w_tile(c):
            return wb_sb[0:5, c * 128 : (c + 1) * 128]

        with tc.tile_pool(name="psum", bufs=1, space="PSUM") as pp:
            psum_zr = pp.tile([128, 16, ZCOLS], fdt)
            psum_mx = pp.tile([128, MC, BLK], fdt)
            psum_hg = [
                pp.tile([128, 8], fdt, name="psum_hg0"),
                pp.tile([128, 8], fdt, name="psum_hg1"),
            ]
            psum_mxa = pp.tile([128, 8], fdt)
            psum_d = pp.tile([1, 1], fdt)

            # ---- init: load everything, zero h ----
            nc.gpsimd.dma_start(out=r_sb, in_=r_d)
            nc.gpsimd.dma_start(out=wb_sb, in_=wb_d)
            nc.gpsimd.dma_start(out=xt_sb, in_=xt_d)
            nc.gpsimd.dma_start(out=dw_sb, in_=dw_d)
            nc.vector.memset(hb[0], 0.0)

            def emit_group(psum_ap, h_ap, c, tail_mm=None):
                """One consecutive accumulation group: 8 R-tile MMs (+ tail)."""
                for k in range(KC):
                    nc.tensor.matmul(
                        psum_ap, r_tile(k, c), h_ap[:, k : k + 1],
                        start=(k == 0), stop=(tail_mm is None and k == KC - 1),
                        skip_group_check=True,
                    )
                if tail_mm is not None:
                    w_ap, x_ap = tail_mm
                    nc.tensor.matmul(psum_ap, w_ap, x_ap,
                                     start=False, stop=True,
                                     skip_group_check=True)

            def emit_chain(h_prev, h_next, bt, psum_h, zr_in, mxh_ap):
                if zr_in is not None:
                    nc.scalar.activation(zr_s, zr_in, AF.Sigmoid)
                nc.vector.tensor_tensor(t1, zr_s[:, 8:16], psum_h[:, :], op=OP.mult)
                nc.vector.tensor_tensor(t2, t1, mxh_ap, op=OP.add)
                nc.scalar.activation(hh, t2, AF.Tanh)
                nc.vector.tensor_tensor(dd, h_prev, hh, op=OP.subtract)
                nc.vector.tensor_tensor(ee, dd, zr_s[:, 0:8], op=OP.mult)
                nc.vector.tensor_tensor(h_next, ee, hh, op=OP.add)

            # ---- warmup: 4096 steps in blocks of BLK ----
            with tc.For_i(0, TW, BLK) as i:
                xblk = xt_sb[0:5, ds(i, BLK)]
                for c in range(MC):
                    nc.tensor.matmul(
                        psum_mx[:, c, 0:BLK], w_tile(c), xblk,
                        start=True, stop=True, skip_group_check=True,
                    )
                nc.vector.tensor_copy(mx_sb[:, :, :], psum_mx[:, :, :])
                for bt in range(BLK):
                    par = bt % 2
                    h_ap = hb[par]
                    # zr groups first: sigmoid overlaps the h-gate matmuls
                    for c in range(16):
                        emit_group(psum_zr[:, c, bt : bt + 1], h_ap, c)
                    nc.vector.tensor_tensor(
                        zr_pre, psum_zr[:, :, bt], mx_sb[:, 0:16, bt], op=OP.add
                    )
                    nc.scalar.activation(zr_s, zr_pre, AF.Sigmoid)
                    for c in range(16, MC):
                        emit_group(psum_hg[par][:, c - 16 : c - 15], h_ap, c)
                    emit_chain(h_ap, hb[1 - par], bt, psum_hg[par],
                               None, mx_sb[:, 16:24, bt])

            # ---- autoregressive: 399 steps ----
            def emit_ar_step(bt, xcol, jcol):
                par = bt % 2
                h_ap = hb[par]
                # dense matvec on h_prev -> pred
                for k in range(KC):
                    nc.tensor.matmul(
                        psum_d[:, :], dw_sb[:, k : k + 1], h_ap[:, k : k + 1],
                        start=(k == 0), stop=(k == KC - 1), skip_group_check=True,
                    )
                nc.scalar.activation(pr[0:1, jcol], psum_d[:, :], AF.Sigmoid,
                                     bias=dense_bias)
                # feed pred back as input feature (stored on partition 0)
                nc.vector.tensor_copy(xt_sb[0:1, xcol], pr[0:1, jcol])
                xin = xt_sb[0:5, xcol]
                # h-gate R groups first (no pred dependency) ...
                for c in range(16, MC):
                    emit_group(psum_hg[par][:, c - 16 : c - 15], h_ap, c)
                # ... then zr groups, each ending with the K=5 x-part matmul
                for c in range(16):
                    emit_group(psum_zr[:, c, bt : bt + 1], h_ap, c,
                               tail_mm=(w_tile(c), xin))
                # h-gate x-part (atomic single-MM groups)
                for c in range(16, MC):
                    nc.tensor.matmul(
                        psum_mxa[:, c - 16 : c - 15], w_tile(c), xin,
                        start=True, stop=True, skip_group_check=True,
                    )
                emit_chain(h_ap, hb[1 - par], bt, psum_hg[par],
                           psum_zr[:, :, bt], psum_mxa[:, :])

            with tc.For_i(0, NARB * UARB, UARB) as i:
                for bt in range(UARB):
                    emit_ar_step(bt, ds(i + (AR0 + bt), 1), ds(i + bt, 1))
            for bt in range(ARTAIL):
                j = NARB * UARB + bt
                emit_ar_step(bt, slice(AR0 + j, AR0 + j + 1), slice(j, j + 1))

            # final pred (399) from the last hidden state
            h_fin = hb[ARTAIL % 2]
            for k in range(KC):
                nc.tensor.matmul(
                    psum_d[:, :], dw_sb[:, k : k + 1], h_fin[:, k : k + 1],
                    start=(k == 0), stop=(k == KC - 1), skip_group_check=True,
                )
            nc.scalar.activation(pr[0:1, OUT_STEPS - 1 : OUT_STEPS], psum_d[:, :],
                                 AF.Sigmoid, bias=dense_bias)
            nc.sync.dma_start(out=out_d, in_=pr)

    nc.compile()
    return nc


def _prep_inputs(inputs, kernel_w, recurrent_kernel, bias, dense_w, np_wdt):
    x = np.asarray(inputs, np.float32)[0]                       # [4496, 4]
    K = np.asarray(kernel_w, np.float32)                        # [4, 3072]
    R = np.asarray(recurrent_kernel, np.float32)                # [1024, 3072]
    B = np.asarray(bias, np.float32)                            # [2, 3072]
    dw = np.asarray(dense_w, np.float32).reshape(UNITS)         # [1024]

    rt = np.ascontiguousarray(
        R.reshape(KC, 128, MC, 128).transpose(1, 0, 2, 3).reshape(128, -1)
    )
    # feature order permuted so the fed-back prediction sits on partition 0:
    # rows = [feat3 (SoC / pred), feat0, feat1, feat2, const-1]
    perm = [3, 0, 1, 2]
    wb = np.zeros((5, U3), np.float32)
    wb[0:F] = K[perm]
    wb[4, : 2 * UNITS] = B[0, : 2 * UNITS] + B[1, : 2 * UNITS]  # z,r biases
    wb[4, 2 * UNITS :] = B[0, 2 * UNITS :]                      # h-gate input bias
    xt = np.concatenate([x.T[perm], np.ones((1, SEQ), np.float32)],
                        axis=0)                                  # [5, 4496]
    dsb = np.ascontiguousarray(dw.reshape(KC, 128).T)           # [128, 8]

    return (rt.astype(np_wdt), wb.astype(np_wdt), xt.astype(np_wdt),
            dsb.astype(np_wdt))


def _make_runner(nc):
    """One-time jit of the bass program; returns in_names and callable.

    Mirrors concourse.bass2jax.run_bass_via_pjrt but caches the jitted body so
    repeated calls skip re-lowering the 12k-instruction module.
    """
    import jax
    import concourse.mybir as mybir
    from concourse import bass2jax

    bass2jax.install_neuronx_cc_hook()
    partition_name = nc.partition_id_tensor.name if nc.partition_id_tensor else None
    in_names, out_names, out_avals, zero_outs = [], [], [], []
    for alloc in nc.m.functions[0].allocations:
        if not isinstance(alloc, mybir.MemoryLocationSet):
            continue
        name = alloc.memorylocations[0].name
        if alloc.kind == "ExternalInput":
            if name != partition_name:
                in_names.append(name)
        elif alloc.kind == "ExternalOutput":
            shape = tuple(alloc.tensor_shape)
            dtype = mybir.dt.np(alloc.dtype)
            out_names.append(name)
            out_avals.append(jax.core.ShapedArray(shape, dtype))
            zero_outs.append(np.zeros(shape, dtype))
    n_params = len(in_names)
    all_names = in_names + out_names
    if partition_name is not None:
        all_names = all_names + [partition_name]
    donate = tuple(range(n_params, n_params + len(out_names)))

    def _body(*args):
        operands = list(args)
        if partition_name is not None:
            operands.append(bass2jax.partition_id_tensor())
        outs = bass2jax._bass_exec_p.bind(
            *operands,
            out_avals=tuple(out_avals),
            in_names=tuple(all_names),
            out_names=tuple(out_names),
            lowering_input_output_aliases=(),
            sim_require_finite=True,
            sim_require_nnan=True,
            nc=nc,
        )
        return tuple(outs)

    jitted = jax.jit(_body, donate_argnums=donate, keep_unused=True)

    def run(in_map):
        args = [np.asarray(in_map[n]) for n in in_names]
        args += [np.zeros_like(z) for z in zero_outs]
        outs = jitted(*args)
        return {n: np.asarray(o) for n, o in zip(out_names, outs)}

    return run


def kernel(inputs, kernel, recurrent_kernel, bias, dense_w, dense_b,
           _dt="bf16") -> np.ndarray:
    import hashlib
    import ml_dtypes

    np_wdt = ml_dtypes.bfloat16 if _dt == "bf16" else np.float32
    db = float(np.asarray(dense_b, np.float32).reshape(-1)[0])
    rt, wb, xt, dsb = _prep_inputs(inputs, kernel, recurrent_kernel, bias,
                                   dense_w, np_wdt)
    wkey = hashlib.sha1(
        rt.tobytes() + wb.tobytes() + dsb.tobytes() + str(db).encode()
    ).hexdigest()
    key = (_dt, wkey)
    if key not in _cache:
        nc = _build(_dt, db, rt, wb, dsb)
        _cache[key] = _make_runner(nc)
    run = _cache[key]
    res = run({"xt_t": xt})
    return np.asarray(res["preds"], np.float32).reshape(OUT_STEPS)
